# revision 3
# baseline (speedup 1.0000x reference)
"""Trainium2 Bass kernel for ContinuousIntegratedKoopmanOperator.

reference: odeint(dz/dt = z @ W) sampled at t = DT*[1..T], y0 = x at t[0].
Closed form (time-invariant linear ODE): out[:, j, :] = x @ expm(DT*j*W).

Strategy (v2, delta-fp8 rewrite of the 64.5us fp16 baseline):
  The fp16 baseline was drain-bound (PSUM f32 -> f16 on V+S at ~1 elem/
  lane/cycle for 37us) with stores right behind (~450 GB/s when HAM
  unthrottled).  This version:
    * computes per-step DELTAS d_j = x @ (M^j - M^{j-1}), j=1..63, with
      per-column power-of-2 pre-scaling to unit-ish std; stores them as
      fp8e3 (e3m4) -- halves both drained... no, drains are per-element,
      but halves STORE bytes (8.4MB/core); host decodes + cumsums from
      exact f32 x, so quantization errors random-walk: ~2.5e-3 rel total.
    * matmuls in fp8e4 DoubleRow perf mode (0.5 cyc/row): the doubled
      contraction (2 k-tiles of 128) carries x-quantized + its residual
      (x ~= x8 + r8), killing the fp8-x accuracy loss for free.
    * drains split Vector/Scalar (greedy 2048-col quads incl. bank pads),
      warmup copies on V/S so the first gated drain isn't hit by blocked-
      sem wake latency.
    * trailing dummy matmuls keep the PE busy through the store tail:
      HAM drops to k=4 (DMA crawls) ~3.6us after the PE goes idle.
    * 16 half-tile stores (512KB) so the DMA tail drains wide.
"""
import numpy as np
import ml_dtypes

DT = 0.01
B, D, T = 8192, 128, 64
NCORES = 8
BSH = B // NCORES          # 1024 rows per core
NTILES = BSH // 128        # 8 batch tiles per core
NJ = T - 1                 # j=1..63 on device; j=0 is x itself (host)
CT = NJ * D                # 8064 real output cols per row
UW = 504                   # matmul unit width (fits a 512-col psum bank)
NU_T = CT // UW            # 16 units per tile
NUNITS = NTILES * NU_T     # 128 units
SLOT = 512                 # psum bank width (f32 cols); unit u -> bank u%8
PADW = SLOT * NU_T         # 8192 staged cols per tile (incl 8-col pads)
QUADS = NUNITS // 4        # 32 drain quads (4 banks = 2048 cols each)
MCW = 4 * UW               # 2016-col md load chunks (= 4 units)
DUMW = 17                  # leading PE warmup matmuls
TRAILD = 14                # trailing dummies: hold HAM k=8 through store tail
TSTD = 1.4                 # target per-column std of device outputs (e3m4)

# static drain-quad engine assignment (greedy by finish time)
_DUR = {"V": 2283.0, "S": 1837.0}
ENG_OF, IDX_OF = [], []
_fin = {"V": 0.0, "S": 0.0}
_cnt = {"V": 0, "S": 0}
for _q in range(QUADS):
    _e = min(("S", "V"), key=lambda e: _fin[e] + _DUR[e])
    _fin[_e] += _DUR[_e]
    _cnt[_e] += 1
    ENG_OF.append(_e)
    IDX_OF.append(_cnt[_e])

_CACHE = {}


def _host_tables(W: np.ndarray):
    """float64 delta table -> (md fp8e4 [128, 2*CT] ktile-duplicated,
    s2 f32 [NJ, D] decode scales)."""
    A = DT * W.astype(np.float64)
    M1 = np.eye(D)
    term = np.eye(D)
    for n in range(1, 30):
        term = term @ A / n
        M1 += term
    E = M1 - np.eye(D)
    Dp = np.empty((D, CT), dtype=np.float64)  # scaled deltas, j-major cols
    s2 = np.empty((NJ, D), dtype=np.float32)
    P = np.eye(D)                             # M^{j-1}
    for j in range(1, T):
        Dj = P @ E                            # M^{j-1} (M - I)
        P = P @ M1
        cn = np.linalg.norm(Dj, axis=0) / TSTD
        sc = np.exp2(np.round(np.log2(cn)))
        s2[j - 1] = sc.astype(np.float32)
        Dp[:, (j - 1) * D:j * D] = Dj / sc[None, :]
    md8 = Dp.astype(ml_dtypes.float8_e4m3)
    md = np.empty((D, 2 * CT), dtype=ml_dtypes.float8_e4m3)
    md[:, :CT] = md8
    md[:, CT:] = md8
    return md, s2


def _build_nc():
    import concourse.bass as bass
    import concourse.mybir as mybir

    f32 = mybir.dt.float32
    f8e4 = mybir.dt.float8e4
    f8e3 = mybir.dt.float8e3
    DR = mybir.MatmulPerfMode.DoubleRow

    nc = bass.Bass(trn_type="TRN2")
    xr_d = nc.dram_tensor("xr", (D, 2 * BSH), f8e4, kind="ExternalInput")
    md_d = nc.dram_tensor("md", (D, 2 * CT), f8e4, kind="ExternalInput")
    out_d = nc.dram_tensor("out8", (BSH, PADW), f8e3, kind="ExternalOutput")

    xr_s = nc.alloc_sbuf_tensor("xr_s", [D, 2, BSH], f8e4)
    md_s = nc.alloc_sbuf_tensor("md_s", [D, 2, CT], f8e4)
    stg = [nc.alloc_sbuf_tensor(f"stg{i}", [128, PADW], f8e3) for i in range(NTILES)]
    scr = nc.alloc_sbuf_tensor("scr", [128, 6144], f8e3)  # V/S warmup scratch
    psum = nc.alloc_psum_tensor("acc", [128, 8 * SLOT], f32)

    s_ld = nc.alloc_semaphore("s_ld")
    s_mm = nc.alloc_semaphore("s_mm")
    s_dv = nc.alloc_semaphore("s_dv")
    s_da = nc.alloc_semaphore("s_da")
    s_out = nc.alloc_semaphore("s_out")
    s_boot = nc.alloc_semaphore("s_boot")
    all_sems = [s_ld, s_mm, s_dv, s_da, s_out, s_boot]
    nums = sorted(s.num for s in all_sems)
    assert nums == list(range(nums[0], nums[-1] + 1)), "sems not contiguous"
    sem_range = range(nums[0], nums[-1] + 1)
    nc.gpsimd.dma_reset(sem_range)

    def quad_wait(eng, q):
        eng.wait_ge(s_dv if ENG_OF[q] == "V" else s_da, IDX_OF[q])

    with nc.Block() as block:
        @block.sync
        def _(sync):
            sync.sem_clear(sem_range)
            sync.nop().then_inc(s_boot, 1)
            sync.dma_start(out=xr_s[:, :, :], in_=xr_d[:, :]).then_inc(s_ld, 16)
            for c in range(4):
                for k in range(2):
                    sync.dma_start(
                        out=md_s[:, k:k + 1, c * MCW:(c + 1) * MCW],
                        in_=md_d[:, k * CT + c * MCW:k * CT + (c + 1) * MCW],
                    ).then_inc(s_ld, 16)
            for st in range(2 * NTILES):
                q_hi = 2 * st + 1
                cv = sum(1 for q in range(q_hi + 1) if ENG_OF[q] == "V")
                ca = (q_hi + 1) - cv
                if cv:
                    sync.wait_ge(s_dv, cv)
                if ca:
                    sync.wait_ge(s_da, ca)
                t, h = st // 2, st % 2
                sync.dma_start(
                    out=out_d[t * 128:(t + 1) * 128, h * 4096:(h + 1) * 4096],
                    in_=stg[t][:, h * 4096:(h + 1) * 4096],
                ).then_inc(s_out, 16)
            sync.wait_ge(s_out, 16 * 2 * NTILES)

        @block.scalar
        def _(scalar):
            # warmup: keep ACT hot so the first gated drain isn't blocked-idle
            scalar.copy(out=scr[:, 3072:4608], in_=scr[:, 4608:6144])
            scalar.copy(out=scr[:, 3072:4608], in_=scr[:, 4608:6144])
            scalar.wait_ge(s_boot, 1)
            for q in range(QUADS):
                if ENG_OF[q] != "S":
                    continue
                scalar.wait_ge(s_mm, 4 * q + 4)
                po = (4 * q % 8) * SLOT
                scalar.copy(out=stg[q // 4][:, (q % 4) * 2048:(q % 4 + 1) * 2048],
                            in_=psum[:, po:po + 2048]).then_inc(s_da, 1)

        @block.vector
        def _(vector):
            vector.tensor_copy(out=scr[:, 0:1536], in_=scr[:, 1536:3072])
            vector.tensor_copy(out=scr[:, 0:1536], in_=scr[:, 1536:3072])
            vector.wait_ge(s_boot, 1)
            for q in range(QUADS):
                if ENG_OF[q] != "V":
                    continue
                vector.wait_ge(s_mm, 4 * q + 4)
                po = (4 * q % 8) * SLOT
                vector.tensor_copy(out=stg[q // 4][:, (q % 4) * 2048:(q % 4 + 1) * 2048],
                                   in_=psum[:, po:po + 2048]).then_inc(s_dv, 1)

        @block.tensor
        def _(tensor):
            # leading dummies: warm the PE p-state + HAM through the NEFF
            # preamble/load window; 512-wide so every psum col (incl pads)
            # is initialized before any drain reads it.
            for k in range(DUMW):
                tensor.matmul(psum[:, (k % 8) * SLOT:(k % 8) * SLOT + SLOT],
                              xr_s[:, :, 0:128], md_s[:, :, 0:SLOT],
                              start=True, stop=True, perf_mode=DR)
            tensor.wait_ge(s_boot, 1)
            for u in range(NUNITS):
                t, b = u // NU_T, u % NU_T
                if t == 0 and b % 4 == 0:
                    tensor.wait_ge(s_ld, 16 * (3 + 2 * (b // 4)))
                if u >= 8 and u % 4 == 0:
                    quad_wait(tensor, (u - 8) // 4)
                po = (u % 8) * SLOT
                mm = tensor.matmul(psum[:, po:po + UW],
                                   xr_s[:, :, t * 128:(t + 1) * 128],
                                   md_s[:, :, b * UW:(b + 1) * UW],
                                   start=True, stop=True, perf_mode=DR)
                mm.then_inc(s_mm, 1)
            # trailing dummies: HAM drops DMA to k=4 ~3.6us after PE idles,
            # which crawls the store tail. Keep streaming garbage matmuls.
            for k in range(TRAILD):
                if k < 8 and k % 4 == 0:
                    quad_wait(tensor, 30 + k // 4)
                tensor.matmul(psum[:, (k % 8) * SLOT:(k % 8) * SLOT + SLOT],
                              xr_s[:, :, 0:128], md_s[:, :, 0:SLOT],
                              start=True, stop=True, perf_mode=DR)

    return nc


def _prep_inputs(x: np.ndarray, md8: np.ndarray):
    """Per-core input maps: xr = [p][ktile*BSH + m] fp8e4 with ktile0 = x8^T,
    ktile1 = r8^T (r = x - x8)."""
    maps = []
    for c in range(NCORES):
        xc = x[c * BSH:(c + 1) * BSH]                       # (BSH, D) f32
        x8 = xc.astype(ml_dtypes.float8_e4m3)
        r = xc - x8.astype(np.float32)
        r8 = r.astype(ml_dtypes.float8_e4m3)
        xr = np.empty((D, 2 * BSH), dtype=ml_dtypes.float8_e4m3)
        xr[:, :BSH] = x8.T
        xr[:, BSH:] = r8.T
        maps.append({"xr": np.ascontiguousarray(xr), "md": md8})
    return maps


def run_on_device(x: np.ndarray, md8: np.ndarray, s2: np.ndarray, trace: bool = False):
    from concourse.bass_utils import run_bass_kernel_spmd

    if "nc" not in _CACHE:
        _CACHE["nc"] = _build_nc()
    nc = _CACHE["nc"]

    in_maps = _prep_inputs(x, md8)
    res = run_bass_kernel_spmd(nc, in_maps, core_ids=list(range(NCORES)), trace=trace)
    out = np.empty((B, T, D), dtype=np.float32)
    for c in range(NCORES):
        xc = x[c * BSH:(c + 1) * BSH].astype(np.float32)
        raw = res.results[c]["out8"].astype(np.float32)     # (BSH, PADW)
        d = raw.reshape(BSH, NU_T, SLOT)[:, :, :UW].reshape(BSH, NJ, D)
        d *= s2[None, :, :]
        out[c * BSH:(c + 1) * BSH, 0] = xc
        out[c * BSH:(c + 1) * BSH, 1:] = xc[:, None, :] + np.cumsum(d, axis=1)
    return out, res


def kernel(x, W, T):
    x = np.asarray(x, dtype=np.float32)
    W = np.asarray(W, dtype=np.float32)
    assert int(T) == 64 and x.shape == (B, D) and W.shape == (D, D)
    md8, s2 = _host_tables(W)
    out, _ = run_on_device(x, md8, s2, trace=False)
    return out
